# revision 15
# baseline (speedup 1.0000x reference)
"""Product-quantization kernel for Trainium2, data-parallel over 8 NeuronCores.

Per core (N/8 = 16384 vectors, 128 tiles of 128 vectors):
  - distances via PE matmuls: metric m[n, k, c] = v_k(n).cb[k,c] - |cb[k,c]|^2/2
    (argmax_c m  ==  argmin_c ||v_k - cb_kc||^2), bundled block-diagonally
    (k=0..3 | k=4..7).  fp16 hi/lo split: pass 1 = vh.ch + bias (K=66, two
    ones-rows carry the bias hi/lo), pass 2 = vl.ch + vh.cl stacked (K=128),
    accumulated in fp32 PSUM -> fp32-class accuracy at fp16 matmul rate.
  - argmax via DVE reduce_max (per-k maxima) + max_index (first index of the
    max within each 256-wide segment -> matches jnp.argmin tie-breaking).
  - reconstruction via one GPSIMD ap_gather per tile over a replicated
    codebook-transpose table; the 16x16-block-transposed output layout is
    unshuffled on the host.
"""

import sys

if "/opt/trn_rl_repo" not in sys.path:
    sys.path.insert(0, "/opt/trn_rl_repo")

import numpy as np

N = 131072
D = 128
K = 8
C = 256
SD = 16
NCORES = 8
NPER = N // NCORES  # 16384
TILE_N = 128

MM_MODE = "fp16hilo"  # fp32 | fp16hilo
SBUF_SCAN = False  # ACT-evacuate PSUM, DVE scans read SBUF
MERGED_GATHER = False

_BUILD_CACHE = {}


def _build_module(n_per, mm_mode=None, sbuf_scan=None, merged_gather=None):
    """Build the Bass module (shared by all 8 cores, SPMD)."""
    from contextlib import ExitStack

    import concourse.bacc as bacc
    import concourse.mybir as mybir
    import concourse.tile as tile

    if mm_mode is None:
        mm_mode = MM_MODE
    if sbuf_scan is None:
        sbuf_scan = SBUF_SCAN
    if merged_gather is None:
        merged_gather = MERGED_GATHER

    f32 = mybir.dt.float32
    f16 = mybir.dt.float16
    u16 = mybir.dt.uint16
    i16 = mybir.dt.int16

    ntiles = n_per // TILE_N
    nc = bacc.Bacc("TRN2", target_bir_lowering=False, debug=False)

    fp16 = mm_mode == "fp16hilo"
    if fp16:
        # vth rows: 0-63 vh dims 0-63, 64-65 ones, 66-129 vh dims 64-127, 130-131 ones
        vth = nc.dram_tensor("vth", [132, n_per], f16, kind="ExternalInput").ap()
        # vtc rows: 0-63 vl dims 0-63, 64-127 vh dims 0-63, 128-191 vl 64-127, 192-255 vh 64-127
        vtc = nc.dram_tensor("vtc", [256, n_per], f16, kind="ExternalInput").ap()
        wha = nc.dram_tensor("wha", [66, 1024], f16, kind="ExternalInput").ap()
        whb = nc.dram_tensor("whb", [66, 1024], f16, kind="ExternalInput").ap()
        wca = nc.dram_tensor("wca", [128, 1024], f16, kind="ExternalInput").ap()
        wcb = nc.dram_tensor("wcb", [128, 1024], f16, kind="ExternalInput").ap()
    else:
        # vt rows: 0..63 dims 0..63, 64 ones, 65..128 dims 64..127, 129 ones
        vt = nc.dram_tensor("vt", [130, n_per], f32, kind="ExternalInput").ap()
        wa = nc.dram_tensor("wa", [65, 1024], f32, kind="ExternalInput").ap()
        wb = nc.dram_tensor("wb", [65, 1024], f32, kind="ExternalInput").ap()
    cbt = nc.dram_tensor("cbt", [128, 2048], f32, kind="ExternalInput").ap()
    koff = nc.dram_tensor("koff", [128, 8], u16, kind="ExternalInput").ap()
    idx_dev = nc.dram_tensor("idx_dev", [n_per, 64], u16, kind="ExternalOutput").ap()
    rec_dev = nc.dram_tensor(
        "rec_dev", [ntiles, 128, 128], f32, kind="ExternalOutput"
    ).ap()

    with tile.TileContext(nc) as tc, ExitStack() as ctx:
        const = ctx.enter_context(tc.tile_pool(name="const", bufs=1))
        if fp16:
            wha_s = const.tile([66, 1024], f16, tag="wha")
            nc.sync.dma_start(wha_s[:], wha[:])
            whb_s = const.tile([66, 1024], f16, tag="whb")
            nc.sync.dma_start(whb_s[:], whb[:])
            wca_s = const.tile([128, 1024], f16, tag="wca")
            nc.sync.dma_start(wca_s[:], wca[:])
            wcb_s = const.tile([128, 1024], f16, tag="wcb")
            nc.sync.dma_start(wcb_s[:], wcb[:])
        else:
            wa_s = const.tile([65, 1024], f32, tag="wa")
            nc.sync.dma_start(wa_s[:], wa[:])
            wb_s = const.tile([65, 1024], f32, tag="wb")
            nc.sync.dma_start(wb_s[:], wb[:])
        cbt_s = const.tile([128, 2048], f32, tag="cbt")
        nc.sync.dma_start(cbt_s[:], cbt[:])
        koff_s = const.tile([128, 8], u16, tag="koff")
        nc.sync.dma_start(koff_s[:], koff[:])

        vpool = ctx.enter_context(tc.tile_pool(name="v", bufs=4))
        dpoolA = ctx.enter_context(tc.tile_pool(name="distA", bufs=2, space="PSUM"))
        dpoolB = ctx.enter_context(tc.tile_pool(name="distB", bufs=2, space="PSUM"))
        spool = ctx.enter_context(tc.tile_pool(name="small", bufs=4))
        outpool = ctx.enter_context(tc.tile_pool(name="outs", bufs=4))
        if sbuf_scan:
            espool = ctx.enter_context(tc.tile_pool(name="evac", bufs=2))

        for t in range(ntiles):
            n0 = t * TILE_N
            sl = slice(n0, n0 + 128)
            if fp16:
                vha = vpool.tile([66, 128], f16, tag="vha")
                nc.sync.dma_start(vha[:], vth[0:66, sl])
                vhb = vpool.tile([66, 128], f16, tag="vhb")
                nc.sync.dma_start(vhb[:], vth[66:132, sl])
                vca = vpool.tile([128, 128], f16, tag="vca")
                nc.sync.dma_start(vca[:], vtc[0:128, sl])
                vcb = vpool.tile([128, 128], f16, tag="vcb")
                nc.sync.dma_start(vcb[:], vtc[128:256, sl])
            else:
                va = vpool.tile([65, 128], f32, tag="va")
                nc.sync.dma_start(va[:], vt[0:65, sl])
                vb = vpool.tile([65, 128], f32, tag="vb")
                nc.sync.dma_start(vb[:], vt[65:130, sl])

            dA = dpoolA.tile([128, 1024], f32, tag="dA")
            dB = dpoolB.tile([128, 1024], f32, tag="dB")
            for j in (0, 1):
                cs = slice(j * 512, (j + 1) * 512)
                if fp16:
                    nc.tensor.matmul(
                        dA[:, cs], lhsT=vha[:], rhs=wha_s[:, cs], start=True, stop=False
                    )
                    nc.tensor.matmul(
                        dA[:, cs], lhsT=vca[:], rhs=wca_s[:, cs], start=False, stop=True
                    )
                    nc.tensor.matmul(
                        dB[:, cs], lhsT=vhb[:], rhs=whb_s[:, cs], start=True, stop=False
                    )
                    nc.tensor.matmul(
                        dB[:, cs], lhsT=vcb[:], rhs=wcb_s[:, cs], start=False, stop=True
                    )
                else:
                    nc.tensor.matmul(
                        dA[:, cs], lhsT=va[:], rhs=wa_s[:, cs], start=True, stop=True
                    )
                    nc.tensor.matmul(
                        dB[:, cs], lhsT=vb[:], rhs=wb_s[:, cs], start=True, stop=True
                    )

            if sbuf_scan:
                sA = espool.tile([128, 1024], f32, tag="sA")
                nc.scalar.copy(sA[:], dA[:])
                sB = espool.tile([128, 1024], f32, tag="sB")
                nc.scalar.copy(sB[:], dB[:])
                srcA, srcB = sA, sB
            else:
                srcA, srcB = dA, dB

            m8 = spool.tile([128, 8], f32, tag="m8")
            nc.vector.reduce_max(
                m8[:, 0:4],
                srcA[:].rearrange("p (k c) -> p k c", c=256),
                axis=mybir.AxisListType.X,
                opt_input=False,
                opt_output=False,
            )
            nc.vector.reduce_max(
                m8[:, 4:8],
                srcB[:].rearrange("p (k c) -> p k c", c=256),
                axis=mybir.AxisListType.X,
                opt_input=False,
                opt_output=False,
            )

            idx64 = spool.tile([128, 64], u16, tag="idx")
            for k in range(K):
                s_t = srcA if k < 4 else srcB
                seg = s_t[:, (k % 4) * 256 : (k % 4) * 256 + 256]
                # in_max = this segment's max broadcast into all 8 match
                # slots (free stride 0); slot 0 then always holds the first
                # occurrence within THIS segment. Using all 8 per-k maxima
                # instead is subtly broken: a bit-equal max in an earlier
                # segment consumes the match position and yields -1 here.
                m_b = m8[:, k : k + 1].broadcast_to((128, 8))
                nc.vector.max_index(idx64[:, k * 8 : (k + 1) * 8], m_b, seg)

            slab = outpool.tile([128, 128], f32, tag="slab")
            if merged_gather:
                idxw = spool.tile([128, 8], u16, tag="idxw")
                nc.vector.tensor_tensor(
                    idxw[:], idx64[:, 0:64:8], koff_s[:], op=mybir.AluOpType.add
                )
                nc.gpsimd.ap_gather(
                    slab[:],
                    cbt_s[:],
                    idxw[:].bitcast(i16),
                    channels=128,
                    num_elems=2048,
                    d=1,
                    num_idxs=128,
                )
            else:
                for k in range(K):
                    nc.gpsimd.ap_gather(
                        slab[:, k * 16 : (k + 1) * 16],
                        cbt_s[:, k * 256 : (k + 1) * 256],
                        idx64[:, k * 8 : k * 8 + 1].bitcast(i16),
                        channels=128,
                        num_elems=256,
                        d=1,
                        num_idxs=16,
                    )

            nc.sync.dma_start(idx_dev[sl, :], idx64[:])
            nc.sync.dma_start(rec_dev[t], slab[:])

    nc.compile()
    return nc


def _host_inputs(vector, codebook, mm_mode=None):
    """Host-side input staging (layout / constant / precision-split prep)."""
    if mm_mode is None:
        mm_mode = MM_MODE
    cb = np.asarray(codebook, dtype=np.float32)
    v = np.asarray(vector, dtype=np.float32)

    csq_half = -0.5 * (cb ** 2).sum(axis=-1, dtype=np.float32)

    # cbt[16g + j, k*256 + c] = cb[k, c, j]  (independent of group g)
    tmp = cb.transpose(2, 0, 1).reshape(16, K * C)  # [j, (k c)]
    cbt = np.ascontiguousarray(np.tile(tmp, (8, 1)))  # [128, 2048]
    koff = np.broadcast_to(
        (np.arange(8, dtype=np.uint16) * 256)[None], (128, 8)
    ).copy()

    ins = {"cbt": cbt, "koff": koff}

    if mm_mode == "fp16hilo":
        ch = cb.astype(np.float16)
        cl = (cb - ch.astype(np.float32)).astype(np.float16)
        bh = csq_half.astype(np.float16)
        bl = (csq_half - bh.astype(np.float32)).astype(np.float16)

        wha = np.zeros((66, 1024), dtype=np.float16)
        whb = np.zeros((66, 1024), dtype=np.float16)
        wca = np.zeros((128, 1024), dtype=np.float16)
        wcb = np.zeros((128, 1024), dtype=np.float16)
        for k in range(4):
            cseg = slice(k * 256, (k + 1) * 256)
            rows = slice(k * 16, (k + 1) * 16)
            wha[rows, cseg] = ch[k].T
            wha[64, cseg] = bh[k]
            wha[65, cseg] = bl[k]
            wca[rows, cseg] = ch[k].T  # pairs with vl
            wca[64 + k * 16 : 64 + (k + 1) * 16, cseg] = cl[k].T  # pairs with vh
        for k in range(4, 8):
            kk = k - 4
            cseg = slice(kk * 256, (kk + 1) * 256)
            rows = slice(kk * 16, (kk + 1) * 16)
            whb[rows, cseg] = ch[k].T
            whb[64, cseg] = bh[k]
            whb[65, cseg] = bl[k]
            wcb[rows, cseg] = ch[k].T
            wcb[64 + kk * 16 : 64 + (kk + 1) * 16, cseg] = cl[k].T
        ins.update(wha=wha, whb=whb, wca=wca, wcb=wcb)

        vh = v.astype(np.float16)
        vl = (v - vh.astype(np.float32)).astype(np.float16)
        per_core = []
        for core in range(NCORES):
            s = slice(core * NPER, (core + 1) * NPER)
            vhT = vh[s].T  # [128, NPER] fp16
            vlT = vl[s].T
            vth = np.empty((132, NPER), dtype=np.float16)
            vth[0:64] = vhT[0:64]
            vth[64:66] = 1.0
            vth[66:130] = vhT[64:128]
            vth[130:132] = 1.0
            vtc = np.empty((256, NPER), dtype=np.float16)
            vtc[0:64] = vlT[0:64]
            vtc[64:128] = vhT[0:64]
            vtc[128:192] = vlT[64:128]
            vtc[192:256] = vhT[64:128]
            per_core.append({"vth": vth, "vtc": vtc})
    else:
        wa = np.zeros((65, 1024), dtype=np.float32)
        wb = np.zeros((65, 1024), dtype=np.float32)
        for k in range(4):
            wa[k * 16 : (k + 1) * 16, k * 256 : (k + 1) * 256] = cb[k].T
            wa[64, k * 256 : (k + 1) * 256] = csq_half[k]
        for k in range(4, 8):
            kk = k - 4
            wb[kk * 16 : (kk + 1) * 16, kk * 256 : (kk + 1) * 256] = cb[k].T
            wb[64, kk * 256 : (kk + 1) * 256] = csq_half[k]
        ins.update(wa=wa, wb=wb)
        per_core = []
        for core in range(NCORES):
            vc = v[core * NPER : (core + 1) * NPER]
            vte = np.empty((130, NPER), dtype=np.float32)
            vte[0:64] = vc.T[0:64]
            vte[64] = 1.0
            vte[65:129] = vc.T[64:128]
            vte[129] = 1.0
            per_core.append({"vt": vte})
    return ins, per_core


def _decode_outputs(results, n_per):
    ntiles = n_per // TILE_N
    idx_parts = []
    rec_parts = []
    for r in results:
        idx64 = r["idx_dev"]  # [n_per, 64] u16
        idx = idx64[:, ::8].astype(np.int32)  # [n_per, 8]
        idx_parts.append(idx)
        rd = r["rec_dev"].reshape(ntiles, 8, 16, 8, 16)  # [t, g, j, k, i]
        rec = rd.transpose(0, 1, 4, 3, 2).reshape(n_per, 128)
        rec_parts.append(rec)
    indices = np.concatenate(idx_parts, axis=0)
    recon = np.concatenate(rec_parts, axis=0)
    return indices, recon


def run_on_device(vector, codebook, n_per=NPER, trace=False, trace_kwargs=None):
    """Shard, run on the 8 NeuronCores, and reassemble. Returns
    ((indices, recon), BassKernelResults)."""
    from concourse.bass_utils import run_bass_kernel_spmd

    key = (n_per, MM_MODE, SBUF_SCAN, MERGED_GATHER)
    if key not in _BUILD_CACHE:
        _BUILD_CACHE[key] = _build_module(n_per)
    nc = _BUILD_CACHE[key]

    shared, per_core = _host_inputs(vector, codebook)
    in_maps = []
    for core in range(NCORES):
        m = dict(shared)
        for name, arr in per_core[core].items():
            m[name] = np.ascontiguousarray(arr[:, :n_per])
        in_maps.append(m)
    res = run_bass_kernel_spmd(
        nc,
        in_maps,
        core_ids=list(range(NCORES)),
        trace=trace,
        **(trace_kwargs or {}),
    )
    indices, recon = _decode_outputs(res.results, n_per)
    return (indices, recon), res


def kernel(vector, codebook):
    (indices, recon), _ = run_on_device(vector, codebook)
    return indices, recon


# revision 16
# speedup vs baseline: 1.1736x; 1.1736x over previous
"""Product-quantization kernel for Trainium2, data-parallel over 8 NeuronCores.

Per core (N/8 = 16384 vectors, 128 tiles of 128 vectors):
  - distances via PE matmuls: metric m[n, k, c] = v_k(n).cb[k,c] - |cb[k,c]|^2/2
    (argmax_c m  ==  argmin_c ||v_k - cb_kc||^2), bundled block-diagonally
    (k=0..3 | k=4..7).  fp16 hi/lo split: pass 1 = vh.ch + bias (K=66, two
    ones-rows carry the bias hi/lo), pass 2 = vl.ch + vh.cl stacked (K=128),
    accumulated in fp32 PSUM -> fp32-class accuracy at fp16 matmul rate.
  - argmax via DVE reduce_max (per-k maxima) + max_index (first index of the
    max within each 256-wide segment -> matches jnp.argmin tie-breaking).
  - reconstruction via one GPSIMD ap_gather per tile over a replicated
    codebook-transpose table; the 16x16-block-transposed output layout is
    unshuffled on the host.
"""

import sys

if "/opt/trn_rl_repo" not in sys.path:
    sys.path.insert(0, "/opt/trn_rl_repo")

import numpy as np

N = 131072
D = 128
K = 8
C = 256
SD = 16
NCORES = 8
NPER = N // NCORES  # 16384
TILE_N = 128

MM_MODE = "fp16hilo"  # fp32 | fp16hilo
SBUF_SCAN = False  # ACT-evacuate PSUM, DVE scans read SBUF
MERGED_GATHER = True

_BUILD_CACHE = {}


def _build_module(n_per, mm_mode=None, sbuf_scan=None, merged_gather=None):
    """Build the Bass module (shared by all 8 cores, SPMD)."""
    from contextlib import ExitStack

    import concourse.bacc as bacc
    import concourse.mybir as mybir
    import concourse.tile as tile

    if mm_mode is None:
        mm_mode = MM_MODE
    if sbuf_scan is None:
        sbuf_scan = SBUF_SCAN
    if merged_gather is None:
        merged_gather = MERGED_GATHER

    f32 = mybir.dt.float32
    f16 = mybir.dt.float16
    u16 = mybir.dt.uint16
    i16 = mybir.dt.int16

    ntiles = n_per // TILE_N
    nc = bacc.Bacc("TRN2", target_bir_lowering=False, debug=False)

    fp16 = mm_mode == "fp16hilo"
    if fp16:
        # vth rows: 0-63 vh dims 0-63, 64-65 ones, 66-129 vh dims 64-127, 130-131 ones
        vth = nc.dram_tensor("vth", [132, n_per], f16, kind="ExternalInput").ap()
        # vtc rows: 0-63 vl dims 0-63, 64-127 vh dims 0-63, 128-191 vl 64-127, 192-255 vh 64-127
        vtc = nc.dram_tensor("vtc", [256, n_per], f16, kind="ExternalInput").ap()
        wha = nc.dram_tensor("wha", [66, 1024], f16, kind="ExternalInput").ap()
        whb = nc.dram_tensor("whb", [66, 1024], f16, kind="ExternalInput").ap()
        wca = nc.dram_tensor("wca", [128, 1024], f16, kind="ExternalInput").ap()
        wcb = nc.dram_tensor("wcb", [128, 1024], f16, kind="ExternalInput").ap()
    else:
        # vt rows: 0..63 dims 0..63, 64 ones, 65..128 dims 64..127, 129 ones
        vt = nc.dram_tensor("vt", [130, n_per], f32, kind="ExternalInput").ap()
        wa = nc.dram_tensor("wa", [65, 1024], f32, kind="ExternalInput").ap()
        wb = nc.dram_tensor("wb", [65, 1024], f32, kind="ExternalInput").ap()
    cbt = nc.dram_tensor("cbt", [128, 2048], f32, kind="ExternalInput").ap()
    koff = nc.dram_tensor("koff", [128, 8], u16, kind="ExternalInput").ap()
    idx_dev = nc.dram_tensor("idx_dev", [n_per, 64], u16, kind="ExternalOutput").ap()
    rec_dev = nc.dram_tensor(
        "rec_dev", [ntiles, 128, 128], f32, kind="ExternalOutput"
    ).ap()

    with tile.TileContext(nc) as tc, ExitStack() as ctx:
        const = ctx.enter_context(tc.tile_pool(name="const", bufs=1))
        if fp16:
            wha_s = const.tile([66, 1024], f16, tag="wha")
            nc.sync.dma_start(wha_s[:], wha[:])
            whb_s = const.tile([66, 1024], f16, tag="whb")
            nc.sync.dma_start(whb_s[:], whb[:])
            wca_s = const.tile([128, 1024], f16, tag="wca")
            nc.sync.dma_start(wca_s[:], wca[:])
            wcb_s = const.tile([128, 1024], f16, tag="wcb")
            nc.sync.dma_start(wcb_s[:], wcb[:])
        else:
            wa_s = const.tile([65, 1024], f32, tag="wa")
            nc.sync.dma_start(wa_s[:], wa[:])
            wb_s = const.tile([65, 1024], f32, tag="wb")
            nc.sync.dma_start(wb_s[:], wb[:])
        cbt_s = const.tile([128, 2048], f32, tag="cbt")
        nc.sync.dma_start(cbt_s[:], cbt[:])
        koff_s = const.tile([128, 8], u16, tag="koff")
        nc.sync.dma_start(koff_s[:], koff[:])

        vpool = ctx.enter_context(tc.tile_pool(name="v", bufs=4))
        dpoolA = ctx.enter_context(tc.tile_pool(name="distA", bufs=2, space="PSUM"))
        dpoolB = ctx.enter_context(tc.tile_pool(name="distB", bufs=2, space="PSUM"))
        spool = ctx.enter_context(tc.tile_pool(name="small", bufs=4))
        outpool = ctx.enter_context(tc.tile_pool(name="outs", bufs=4))
        if sbuf_scan:
            espool = ctx.enter_context(tc.tile_pool(name="evac", bufs=2))

        for t in range(ntiles):
            n0 = t * TILE_N
            sl = slice(n0, n0 + 128)
            if fp16:
                vha = vpool.tile([66, 128], f16, tag="vha")
                nc.sync.dma_start(vha[:], vth[0:66, sl])
                vhb = vpool.tile([66, 128], f16, tag="vhb")
                nc.sync.dma_start(vhb[:], vth[66:132, sl])
                vca = vpool.tile([128, 128], f16, tag="vca")
                nc.sync.dma_start(vca[:], vtc[0:128, sl])
                vcb = vpool.tile([128, 128], f16, tag="vcb")
                nc.sync.dma_start(vcb[:], vtc[128:256, sl])
            else:
                va = vpool.tile([65, 128], f32, tag="va")
                nc.sync.dma_start(va[:], vt[0:65, sl])
                vb = vpool.tile([65, 128], f32, tag="vb")
                nc.sync.dma_start(vb[:], vt[65:130, sl])

            dA = dpoolA.tile([128, 1024], f32, tag="dA")
            dB = dpoolB.tile([128, 1024], f32, tag="dB")
            for j in (0, 1):
                cs = slice(j * 512, (j + 1) * 512)
                if fp16:
                    nc.tensor.matmul(
                        dA[:, cs], lhsT=vha[:], rhs=wha_s[:, cs], start=True, stop=False
                    )
                    nc.tensor.matmul(
                        dA[:, cs], lhsT=vca[:], rhs=wca_s[:, cs], start=False, stop=True
                    )
                    nc.tensor.matmul(
                        dB[:, cs], lhsT=vhb[:], rhs=whb_s[:, cs], start=True, stop=False
                    )
                    nc.tensor.matmul(
                        dB[:, cs], lhsT=vcb[:], rhs=wcb_s[:, cs], start=False, stop=True
                    )
                else:
                    nc.tensor.matmul(
                        dA[:, cs], lhsT=va[:], rhs=wa_s[:, cs], start=True, stop=True
                    )
                    nc.tensor.matmul(
                        dB[:, cs], lhsT=vb[:], rhs=wb_s[:, cs], start=True, stop=True
                    )

            if sbuf_scan:
                sA = espool.tile([128, 1024], f32, tag="sA")
                nc.scalar.copy(sA[:], dA[:])
                sB = espool.tile([128, 1024], f32, tag="sB")
                nc.scalar.copy(sB[:], dB[:])
                srcA, srcB = sA, sB
            else:
                srcA, srcB = dA, dB

            m8 = spool.tile([128, 8], f32, tag="m8")
            nc.vector.reduce_max(
                m8[:, 0:4],
                srcA[:].rearrange("p (k c) -> p k c", c=256),
                axis=mybir.AxisListType.X,
                opt_input=False,
                opt_output=False,
            )
            nc.vector.reduce_max(
                m8[:, 4:8],
                srcB[:].rearrange("p (k c) -> p k c", c=256),
                axis=mybir.AxisListType.X,
                opt_input=False,
                opt_output=False,
            )

            idx64 = spool.tile([128, 64], u16, tag="idx")
            for k in range(K):
                s_t = srcA if k < 4 else srcB
                seg = s_t[:, (k % 4) * 256 : (k % 4) * 256 + 256]
                # in_max = this segment's max broadcast into all 8 match
                # slots (free stride 0); slot 0 then always holds the first
                # occurrence within THIS segment. Using all 8 per-k maxima
                # instead is subtly broken: a bit-equal max in an earlier
                # segment consumes the match position and yields -1 here.
                m_b = m8[:, k : k + 1].broadcast_to((128, 8))
                nc.vector.max_index(idx64[:, k * 8 : (k + 1) * 8], m_b, seg)

            slab = outpool.tile([128, 128], f32, tag="slab")
            if merged_gather:
                idxw = spool.tile([128, 8], u16, tag="idxw")
                nc.vector.tensor_tensor(
                    idxw[:], idx64[:, 0:64:8], koff_s[:], op=mybir.AluOpType.add
                )
                nc.gpsimd.ap_gather(
                    slab[:],
                    cbt_s[:],
                    idxw[:].bitcast(i16),
                    channels=128,
                    num_elems=2048,
                    d=1,
                    num_idxs=128,
                )
            else:
                for k in range(K):
                    nc.gpsimd.ap_gather(
                        slab[:, k * 16 : (k + 1) * 16],
                        cbt_s[:, k * 256 : (k + 1) * 256],
                        idx64[:, k * 8 : k * 8 + 1].bitcast(i16),
                        channels=128,
                        num_elems=256,
                        d=1,
                        num_idxs=16,
                    )

            nc.sync.dma_start(idx_dev[sl, :], idx64[:])
            nc.sync.dma_start(rec_dev[t], slab[:])

    nc.compile()
    return nc


def _host_inputs(vector, codebook, mm_mode=None):
    """Host-side input staging (layout / constant / precision-split prep)."""
    if mm_mode is None:
        mm_mode = MM_MODE
    cb = np.asarray(codebook, dtype=np.float32)
    v = np.asarray(vector, dtype=np.float32)

    csq_half = -0.5 * (cb ** 2).sum(axis=-1, dtype=np.float32)

    # cbt[16g + j, k*256 + c] = cb[k, c, j]  (independent of group g)
    tmp = cb.transpose(2, 0, 1).reshape(16, K * C)  # [j, (k c)]
    cbt = np.ascontiguousarray(np.tile(tmp, (8, 1)))  # [128, 2048]
    koff = np.broadcast_to(
        (np.arange(8, dtype=np.uint16) * 256)[None], (128, 8)
    ).copy()

    ins = {"cbt": cbt, "koff": koff}

    if mm_mode == "fp16hilo":
        ch = cb.astype(np.float16)
        cl = (cb - ch.astype(np.float32)).astype(np.float16)
        bh = csq_half.astype(np.float16)
        bl = (csq_half - bh.astype(np.float32)).astype(np.float16)

        wha = np.zeros((66, 1024), dtype=np.float16)
        whb = np.zeros((66, 1024), dtype=np.float16)
        wca = np.zeros((128, 1024), dtype=np.float16)
        wcb = np.zeros((128, 1024), dtype=np.float16)
        for k in range(4):
            cseg = slice(k * 256, (k + 1) * 256)
            rows = slice(k * 16, (k + 1) * 16)
            wha[rows, cseg] = ch[k].T
            wha[64, cseg] = bh[k]
            wha[65, cseg] = bl[k]
            wca[rows, cseg] = ch[k].T  # pairs with vl
            wca[64 + k * 16 : 64 + (k + 1) * 16, cseg] = cl[k].T  # pairs with vh
        for k in range(4, 8):
            kk = k - 4
            cseg = slice(kk * 256, (kk + 1) * 256)
            rows = slice(kk * 16, (kk + 1) * 16)
            whb[rows, cseg] = ch[k].T
            whb[64, cseg] = bh[k]
            whb[65, cseg] = bl[k]
            wcb[rows, cseg] = ch[k].T
            wcb[64 + kk * 16 : 64 + (kk + 1) * 16, cseg] = cl[k].T
        ins.update(wha=wha, whb=whb, wca=wca, wcb=wcb)

        vh = v.astype(np.float16)
        vl = (v - vh.astype(np.float32)).astype(np.float16)
        per_core = []
        for core in range(NCORES):
            s = slice(core * NPER, (core + 1) * NPER)
            vhT = vh[s].T  # [128, NPER] fp16
            vlT = vl[s].T
            vth = np.empty((132, NPER), dtype=np.float16)
            vth[0:64] = vhT[0:64]
            vth[64:66] = 1.0
            vth[66:130] = vhT[64:128]
            vth[130:132] = 1.0
            vtc = np.empty((256, NPER), dtype=np.float16)
            vtc[0:64] = vlT[0:64]
            vtc[64:128] = vhT[0:64]
            vtc[128:192] = vlT[64:128]
            vtc[192:256] = vhT[64:128]
            per_core.append({"vth": vth, "vtc": vtc})
    else:
        wa = np.zeros((65, 1024), dtype=np.float32)
        wb = np.zeros((65, 1024), dtype=np.float32)
        for k in range(4):
            wa[k * 16 : (k + 1) * 16, k * 256 : (k + 1) * 256] = cb[k].T
            wa[64, k * 256 : (k + 1) * 256] = csq_half[k]
        for k in range(4, 8):
            kk = k - 4
            wb[kk * 16 : (kk + 1) * 16, kk * 256 : (kk + 1) * 256] = cb[k].T
            wb[64, kk * 256 : (kk + 1) * 256] = csq_half[k]
        ins.update(wa=wa, wb=wb)
        per_core = []
        for core in range(NCORES):
            vc = v[core * NPER : (core + 1) * NPER]
            vte = np.empty((130, NPER), dtype=np.float32)
            vte[0:64] = vc.T[0:64]
            vte[64] = 1.0
            vte[65:129] = vc.T[64:128]
            vte[129] = 1.0
            per_core.append({"vt": vte})
    return ins, per_core


def _decode_outputs(results, n_per):
    ntiles = n_per // TILE_N
    idx_parts = []
    rec_parts = []
    for r in results:
        idx64 = r["idx_dev"]  # [n_per, 64] u16
        idx = idx64[:, ::8].astype(np.int32)  # [n_per, 8]
        idx_parts.append(idx)
        rd = r["rec_dev"].reshape(ntiles, 8, 16, 8, 16)  # [t, g, j, k, i]
        rec = rd.transpose(0, 1, 4, 3, 2).reshape(n_per, 128)
        rec_parts.append(rec)
    indices = np.concatenate(idx_parts, axis=0)
    recon = np.concatenate(rec_parts, axis=0)
    return indices, recon


def run_on_device(vector, codebook, n_per=NPER, trace=False, trace_kwargs=None):
    """Shard, run on the 8 NeuronCores, and reassemble. Returns
    ((indices, recon), BassKernelResults)."""
    from concourse.bass_utils import run_bass_kernel_spmd

    key = (n_per, MM_MODE, SBUF_SCAN, MERGED_GATHER)
    if key not in _BUILD_CACHE:
        _BUILD_CACHE[key] = _build_module(n_per)
    nc = _BUILD_CACHE[key]

    shared, per_core = _host_inputs(vector, codebook)
    in_maps = []
    for core in range(NCORES):
        m = dict(shared)
        for name, arr in per_core[core].items():
            m[name] = np.ascontiguousarray(arr[:, :n_per])
        in_maps.append(m)
    res = run_bass_kernel_spmd(
        nc,
        in_maps,
        core_ids=list(range(NCORES)),
        trace=trace,
        **(trace_kwargs or {}),
    )
    indices, recon = _decode_outputs(res.results, n_per)
    return (indices, recon), res


def kernel(vector, codebook):
    (indices, recon), _ = run_on_device(vector, codebook)
    return indices, recon


# revision 17
# speedup vs baseline: 1.1800x; 1.0054x over previous
"""Product-quantization kernel for Trainium2, data-parallel over 8 NeuronCores.

Per core (N/8 = 16384 vectors, 128 tiles of 128 vectors):
  - distances via PE matmuls: metric m[n, k, c] = v_k(n).cb[k,c] - |cb[k,c]|^2/2
    (argmax_c m  ==  argmin_c ||v_k - cb_kc||^2), bundled block-diagonally
    (k=0..3 | k=4..7).  fp16 hi/lo split: pass 1 = vh.ch + bias (K=66, two
    ones-rows carry the bias hi/lo), pass 2 = vl.ch + vh.cl stacked (K=128),
    accumulated in fp32 PSUM -> fp32-class accuracy at fp16 matmul rate.
  - argmax via DVE reduce_max (per-k maxima) + max_index (first index of the
    max within each 256-wide segment -> matches jnp.argmin tie-breaking).
  - reconstruction via one GPSIMD ap_gather per tile over a replicated
    codebook-transpose table; the 16x16-block-transposed output layout is
    unshuffled on the host.
"""

import sys

if "/opt/trn_rl_repo" not in sys.path:
    sys.path.insert(0, "/opt/trn_rl_repo")

import numpy as np

N = 131072
D = 128
K = 8
C = 256
SD = 16
NCORES = 8
NPER = N // NCORES  # 16384
TILE_N = 128

MM_MODE = "fp16hilo"  # fp32 | fp16hilo
SBUF_SCAN = True  # ACT-evacuate PSUM, DVE scans read SBUF
MERGED_GATHER = True

_BUILD_CACHE = {}


def _build_module(n_per, mm_mode=None, sbuf_scan=None, merged_gather=None):
    """Build the Bass module (shared by all 8 cores, SPMD)."""
    from contextlib import ExitStack

    import concourse.bacc as bacc
    import concourse.mybir as mybir
    import concourse.tile as tile

    if mm_mode is None:
        mm_mode = MM_MODE
    if sbuf_scan is None:
        sbuf_scan = SBUF_SCAN
    if merged_gather is None:
        merged_gather = MERGED_GATHER

    f32 = mybir.dt.float32
    f16 = mybir.dt.float16
    u16 = mybir.dt.uint16
    i16 = mybir.dt.int16

    ntiles = n_per // TILE_N
    nc = bacc.Bacc("TRN2", target_bir_lowering=False, debug=False)

    fp16 = mm_mode == "fp16hilo"
    if fp16:
        # vth rows: 0-63 vh dims 0-63, 64-65 ones, 66-129 vh dims 64-127, 130-131 ones
        vth = nc.dram_tensor("vth", [132, n_per], f16, kind="ExternalInput").ap()
        # vtc rows: 0-63 vl dims 0-63, 64-127 vh dims 0-63, 128-191 vl 64-127, 192-255 vh 64-127
        vtc = nc.dram_tensor("vtc", [256, n_per], f16, kind="ExternalInput").ap()
        wha = nc.dram_tensor("wha", [66, 1024], f16, kind="ExternalInput").ap()
        whb = nc.dram_tensor("whb", [66, 1024], f16, kind="ExternalInput").ap()
        wca = nc.dram_tensor("wca", [128, 1024], f16, kind="ExternalInput").ap()
        wcb = nc.dram_tensor("wcb", [128, 1024], f16, kind="ExternalInput").ap()
    else:
        # vt rows: 0..63 dims 0..63, 64 ones, 65..128 dims 64..127, 129 ones
        vt = nc.dram_tensor("vt", [130, n_per], f32, kind="ExternalInput").ap()
        wa = nc.dram_tensor("wa", [65, 1024], f32, kind="ExternalInput").ap()
        wb = nc.dram_tensor("wb", [65, 1024], f32, kind="ExternalInput").ap()
    cbt = nc.dram_tensor("cbt", [128, 2048], f32, kind="ExternalInput").ap()
    koff = nc.dram_tensor("koff", [128, 8], u16, kind="ExternalInput").ap()
    idx_dev = nc.dram_tensor("idx_dev", [n_per, 64], u16, kind="ExternalOutput").ap()
    rec_dev = nc.dram_tensor(
        "rec_dev", [ntiles, 128, 128], f32, kind="ExternalOutput"
    ).ap()

    with tile.TileContext(nc) as tc, ExitStack() as ctx:
        const = ctx.enter_context(tc.tile_pool(name="const", bufs=1))
        if fp16:
            wha_s = const.tile([66, 1024], f16, tag="wha")
            nc.sync.dma_start(wha_s[:], wha[:])
            whb_s = const.tile([66, 1024], f16, tag="whb")
            nc.sync.dma_start(whb_s[:], whb[:])
            wca_s = const.tile([128, 1024], f16, tag="wca")
            nc.sync.dma_start(wca_s[:], wca[:])
            wcb_s = const.tile([128, 1024], f16, tag="wcb")
            nc.sync.dma_start(wcb_s[:], wcb[:])
        else:
            wa_s = const.tile([65, 1024], f32, tag="wa")
            nc.sync.dma_start(wa_s[:], wa[:])
            wb_s = const.tile([65, 1024], f32, tag="wb")
            nc.sync.dma_start(wb_s[:], wb[:])
        cbt_s = const.tile([128, 2048], f32, tag="cbt")
        nc.sync.dma_start(cbt_s[:], cbt[:])
        koff_s = const.tile([128, 8], u16, tag="koff")
        nc.sync.dma_start(koff_s[:], koff[:])

        vpool = ctx.enter_context(tc.tile_pool(name="v", bufs=4))
        dpoolA = ctx.enter_context(tc.tile_pool(name="distA", bufs=2, space="PSUM"))
        dpoolB = ctx.enter_context(tc.tile_pool(name="distB", bufs=2, space="PSUM"))
        spool = ctx.enter_context(tc.tile_pool(name="small", bufs=4))
        outpool = ctx.enter_context(tc.tile_pool(name="outs", bufs=4))
        if sbuf_scan:
            espool = ctx.enter_context(tc.tile_pool(name="evac", bufs=2))

        for t in range(ntiles):
            n0 = t * TILE_N
            sl = slice(n0, n0 + 128)
            if fp16:
                vha = vpool.tile([66, 128], f16, tag="vha")
                nc.sync.dma_start(vha[:], vth[0:66, sl])
                vhb = vpool.tile([66, 128], f16, tag="vhb")
                nc.sync.dma_start(vhb[:], vth[66:132, sl])
                vca = vpool.tile([128, 128], f16, tag="vca")
                nc.sync.dma_start(vca[:], vtc[0:128, sl])
                vcb = vpool.tile([128, 128], f16, tag="vcb")
                nc.sync.dma_start(vcb[:], vtc[128:256, sl])
            else:
                va = vpool.tile([65, 128], f32, tag="va")
                nc.sync.dma_start(va[:], vt[0:65, sl])
                vb = vpool.tile([65, 128], f32, tag="vb")
                nc.sync.dma_start(vb[:], vt[65:130, sl])

            dA = dpoolA.tile([128, 1024], f32, tag="dA")
            dB = dpoolB.tile([128, 1024], f32, tag="dB")
            for j in (0, 1):
                cs = slice(j * 512, (j + 1) * 512)
                if fp16:
                    nc.tensor.matmul(
                        dA[:, cs], lhsT=vha[:], rhs=wha_s[:, cs], start=True, stop=False
                    )
                    nc.tensor.matmul(
                        dA[:, cs], lhsT=vca[:], rhs=wca_s[:, cs], start=False, stop=True
                    )
                    nc.tensor.matmul(
                        dB[:, cs], lhsT=vhb[:], rhs=whb_s[:, cs], start=True, stop=False
                    )
                    nc.tensor.matmul(
                        dB[:, cs], lhsT=vcb[:], rhs=wcb_s[:, cs], start=False, stop=True
                    )
                else:
                    nc.tensor.matmul(
                        dA[:, cs], lhsT=va[:], rhs=wa_s[:, cs], start=True, stop=True
                    )
                    nc.tensor.matmul(
                        dB[:, cs], lhsT=vb[:], rhs=wb_s[:, cs], start=True, stop=True
                    )

            if sbuf_scan:
                sA = espool.tile([128, 1024], f32, tag="sA")
                nc.scalar.copy(sA[:], dA[:])
                sB = espool.tile([128, 1024], f32, tag="sB")
                nc.scalar.copy(sB[:], dB[:])
                srcA, srcB = sA, sB
            else:
                srcA, srcB = dA, dB

            m8 = spool.tile([128, 8], f32, tag="m8")
            nc.vector.reduce_max(
                m8[:, 0:4],
                srcA[:].rearrange("p (k c) -> p k c", c=256),
                axis=mybir.AxisListType.X,
                opt_input=False,
                opt_output=False,
            )
            nc.vector.reduce_max(
                m8[:, 4:8],
                srcB[:].rearrange("p (k c) -> p k c", c=256),
                axis=mybir.AxisListType.X,
                opt_input=False,
                opt_output=False,
            )

            idx64 = spool.tile([128, 64], u16, tag="idx")
            for k in range(K):
                s_t = srcA if k < 4 else srcB
                seg = s_t[:, (k % 4) * 256 : (k % 4) * 256 + 256]
                # in_max = this segment's max broadcast into all 8 match
                # slots (free stride 0); slot 0 then always holds the first
                # occurrence within THIS segment. Using all 8 per-k maxima
                # instead is subtly broken: a bit-equal max in an earlier
                # segment consumes the match position and yields -1 here.
                m_b = m8[:, k : k + 1].broadcast_to((128, 8))
                nc.vector.max_index(idx64[:, k * 8 : (k + 1) * 8], m_b, seg)

            slab = outpool.tile([128, 128], f32, tag="slab")
            if merged_gather:
                idxw = spool.tile([128, 8], u16, tag="idxw")
                nc.vector.tensor_tensor(
                    idxw[:], idx64[:, 0:64:8], koff_s[:], op=mybir.AluOpType.add
                )
                nc.gpsimd.ap_gather(
                    slab[:],
                    cbt_s[:],
                    idxw[:].bitcast(i16),
                    channels=128,
                    num_elems=2048,
                    d=1,
                    num_idxs=128,
                )
            else:
                for k in range(K):
                    nc.gpsimd.ap_gather(
                        slab[:, k * 16 : (k + 1) * 16],
                        cbt_s[:, k * 256 : (k + 1) * 256],
                        idx64[:, k * 8 : k * 8 + 1].bitcast(i16),
                        channels=128,
                        num_elems=256,
                        d=1,
                        num_idxs=16,
                    )

            nc.sync.dma_start(idx_dev[sl, :], idx64[:])
            nc.sync.dma_start(rec_dev[t], slab[:])

    nc.compile()
    return nc


def _host_inputs(vector, codebook, mm_mode=None):
    """Host-side input staging (layout / constant / precision-split prep)."""
    if mm_mode is None:
        mm_mode = MM_MODE
    cb = np.asarray(codebook, dtype=np.float32)
    v = np.asarray(vector, dtype=np.float32)

    csq_half = -0.5 * (cb ** 2).sum(axis=-1, dtype=np.float32)

    # cbt[16g + j, k*256 + c] = cb[k, c, j]  (independent of group g)
    tmp = cb.transpose(2, 0, 1).reshape(16, K * C)  # [j, (k c)]
    cbt = np.ascontiguousarray(np.tile(tmp, (8, 1)))  # [128, 2048]
    koff = np.broadcast_to(
        (np.arange(8, dtype=np.uint16) * 256)[None], (128, 8)
    ).copy()

    ins = {"cbt": cbt, "koff": koff}

    if mm_mode == "fp16hilo":
        ch = cb.astype(np.float16)
        cl = (cb - ch.astype(np.float32)).astype(np.float16)
        bh = csq_half.astype(np.float16)
        bl = (csq_half - bh.astype(np.float32)).astype(np.float16)

        wha = np.zeros((66, 1024), dtype=np.float16)
        whb = np.zeros((66, 1024), dtype=np.float16)
        wca = np.zeros((128, 1024), dtype=np.float16)
        wcb = np.zeros((128, 1024), dtype=np.float16)
        for k in range(4):
            cseg = slice(k * 256, (k + 1) * 256)
            rows = slice(k * 16, (k + 1) * 16)
            wha[rows, cseg] = ch[k].T
            wha[64, cseg] = bh[k]
            wha[65, cseg] = bl[k]
            wca[rows, cseg] = ch[k].T  # pairs with vl
            wca[64 + k * 16 : 64 + (k + 1) * 16, cseg] = cl[k].T  # pairs with vh
        for k in range(4, 8):
            kk = k - 4
            cseg = slice(kk * 256, (kk + 1) * 256)
            rows = slice(kk * 16, (kk + 1) * 16)
            whb[rows, cseg] = ch[k].T
            whb[64, cseg] = bh[k]
            whb[65, cseg] = bl[k]
            wcb[rows, cseg] = ch[k].T
            wcb[64 + kk * 16 : 64 + (kk + 1) * 16, cseg] = cl[k].T
        ins.update(wha=wha, whb=whb, wca=wca, wcb=wcb)

        vh = v.astype(np.float16)
        vl = (v - vh.astype(np.float32)).astype(np.float16)
        per_core = []
        for core in range(NCORES):
            s = slice(core * NPER, (core + 1) * NPER)
            vhT = vh[s].T  # [128, NPER] fp16
            vlT = vl[s].T
            vth = np.empty((132, NPER), dtype=np.float16)
            vth[0:64] = vhT[0:64]
            vth[64:66] = 1.0
            vth[66:130] = vhT[64:128]
            vth[130:132] = 1.0
            vtc = np.empty((256, NPER), dtype=np.float16)
            vtc[0:64] = vlT[0:64]
            vtc[64:128] = vhT[0:64]
            vtc[128:192] = vlT[64:128]
            vtc[192:256] = vhT[64:128]
            per_core.append({"vth": vth, "vtc": vtc})
    else:
        wa = np.zeros((65, 1024), dtype=np.float32)
        wb = np.zeros((65, 1024), dtype=np.float32)
        for k in range(4):
            wa[k * 16 : (k + 1) * 16, k * 256 : (k + 1) * 256] = cb[k].T
            wa[64, k * 256 : (k + 1) * 256] = csq_half[k]
        for k in range(4, 8):
            kk = k - 4
            wb[kk * 16 : (kk + 1) * 16, kk * 256 : (kk + 1) * 256] = cb[k].T
            wb[64, kk * 256 : (kk + 1) * 256] = csq_half[k]
        ins.update(wa=wa, wb=wb)
        per_core = []
        for core in range(NCORES):
            vc = v[core * NPER : (core + 1) * NPER]
            vte = np.empty((130, NPER), dtype=np.float32)
            vte[0:64] = vc.T[0:64]
            vte[64] = 1.0
            vte[65:129] = vc.T[64:128]
            vte[129] = 1.0
            per_core.append({"vt": vte})
    return ins, per_core


def _decode_outputs(results, n_per):
    ntiles = n_per // TILE_N
    idx_parts = []
    rec_parts = []
    for r in results:
        idx64 = r["idx_dev"]  # [n_per, 64] u16
        idx = idx64[:, ::8].astype(np.int32)  # [n_per, 8]
        idx_parts.append(idx)
        rd = r["rec_dev"].reshape(ntiles, 8, 16, 8, 16)  # [t, g, j, k, i]
        rec = rd.transpose(0, 1, 4, 3, 2).reshape(n_per, 128)
        rec_parts.append(rec)
    indices = np.concatenate(idx_parts, axis=0)
    recon = np.concatenate(rec_parts, axis=0)
    return indices, recon


def run_on_device(vector, codebook, n_per=NPER, trace=False, trace_kwargs=None):
    """Shard, run on the 8 NeuronCores, and reassemble. Returns
    ((indices, recon), BassKernelResults)."""
    from concourse.bass_utils import run_bass_kernel_spmd

    key = (n_per, MM_MODE, SBUF_SCAN, MERGED_GATHER)
    if key not in _BUILD_CACHE:
        _BUILD_CACHE[key] = _build_module(n_per)
    nc = _BUILD_CACHE[key]

    shared, per_core = _host_inputs(vector, codebook)
    in_maps = []
    for core in range(NCORES):
        m = dict(shared)
        for name, arr in per_core[core].items():
            m[name] = np.ascontiguousarray(arr[:, :n_per])
        in_maps.append(m)
    res = run_bass_kernel_spmd(
        nc,
        in_maps,
        core_ids=list(range(NCORES)),
        trace=trace,
        **(trace_kwargs or {}),
    )
    indices, recon = _decode_outputs(res.results, n_per)
    return (indices, recon), res


def kernel(vector, codebook):
    (indices, recon), _ = run_on_device(vector, codebook)
    return indices, recon
